# revision 1
# baseline (speedup 1.0000x reference)
"""8-core Trainium2 kernel for the 2-layer LSTM language model.

Sharding (data-parallel, per spec hint): the batch dim (128) is split
across the 8 NeuronCores (16 sequences per core); the ~10MB tied
embedding and the LSTM weights are replicated. Each core embeds its
tokens, runs both LSTM layers, and computes its slice of the logits
via the tied output projection. Outputs are gathered on the host into
the full [128, 50, 10000] array.
"""

import numpy as np

HIDDEN = 256
VOCAB = 10000
BATCH = 128
SEQ = 50
N_CORES = 8

_compiled = None


def _build():
    import jax
    import jax.numpy as jnp
    from jax import lax

    def lstm_layer(x, w_ih, w_hh, b_ih, b_hh):
        B = x.shape[0]
        H = w_hh.shape[1]
        bias = b_ih + b_hh
        xg = jnp.einsum('bsh,gh->sbg', x, w_ih) + bias

        def step(carry, xt):
            h, c = carry
            gates = xt + h @ w_hh.T
            i, f, g, o = jnp.split(gates, 4, axis=-1)
            i = jax.nn.sigmoid(i)
            f = jax.nn.sigmoid(f)
            g = jnp.tanh(g)
            o = jax.nn.sigmoid(o)
            c = f * c + i * g
            h = o * jnp.tanh(c)
            return (h, c), h

        h0 = jnp.zeros((B, H), dtype=x.dtype)
        c0 = jnp.zeros((B, H), dtype=x.dtype)
        (_, _), hs = lax.scan(step, (h0, c0), xg)
        return jnp.swapaxes(hs, 0, 1)

    def per_core(x, emb, w_ih0, w_hh0, b_ih0, b_hh0, w_ih1, w_hh1, b_ih1, b_hh1):
        embs = emb[x]
        h1 = lstm_layer(embs, w_ih0, w_hh0, b_ih0, b_hh0)
        h2 = lstm_layer(h1, w_ih1, w_hh1, b_ih1, b_hh1)
        return jnp.einsum('bsh,vh->bsv', h2, emb)

    fn = jax.pmap(
        per_core,
        axis_name='cores',
        in_axes=(0, None, None, None, None, None, None, None, None, None),
        devices=jax.devices()[:N_CORES],
    )
    return fn


def kernel(x, emb, w_ih0, w_hh0, b_ih0, b_hh0, w_ih1, w_hh1, b_ih1, b_hh1):
    global _compiled
    if _compiled is None:
        _compiled = _build()
    x = np.asarray(x, dtype=np.int32)
    xs = x.reshape(N_CORES, BATCH // N_CORES, SEQ)
    out = _compiled(
        xs,
        np.asarray(emb, dtype=np.float32),
        np.asarray(w_ih0, dtype=np.float32),
        np.asarray(w_hh0, dtype=np.float32),
        np.asarray(b_ih0, dtype=np.float32),
        np.asarray(b_hh0, dtype=np.float32),
        np.asarray(w_ih1, dtype=np.float32),
        np.asarray(w_hh1, dtype=np.float32),
        np.asarray(b_ih1, dtype=np.float32),
        np.asarray(b_hh1, dtype=np.float32),
    )
    out = np.asarray(out)
    return out.reshape(BATCH, SEQ, VOCAB)
